# revision 22
# baseline (speedup 1.0000x reference)
"""Trainium2 Bass kernel for nn_DependencyParserCombinedAttention.

Model: embeddings -> 2-layer BiLSTM (H=512) -> biaffine attention + MLP
score grid [1, 768, 768].

Implementation (SPMD over 8 NeuronCores):
  - Direction split: cores 0-3 forward, 4-7 backward (time-reversed inputs);
    4-way redundancy within a direction. Exchange between directions via a
    pair-group ReduceScatter (out = exactly the partner payload, half the
    bytes of an AllGather) with the self slot zeroed and the own hidden
    scattered into the partner slot by an indirect DMA.
  - LSTM recurrence via Picard sweeps. First N_F8 sweeps run the recurrent
    (Whh) matmuls in fp8e4 DoubleRow perf mode (K=256 per matmul at 0.5
    cyc/col -> 4x fewer PE cycles than fp16) in GS2 pair-hop order (chunks
    {0,1} then {2,3}; within a hop both chunks' ACT/DVE chains overlap); the
    last N_F16 sweeps run fp16 in GS4 order. Early fp8 sweeps store h
    directly as fp8 (shorter producer chain); the transition sweep stores
    fp16+fp8. All input GEMMs and the exchange payload stay fp16.
  - Exchange 0 (layer 0 -> 1) is latency-hidden: h after sweep N-2 is sent
    under the final sweep (ReduceScatter 1), x_pre1's partner half is built
    from it and the first L1 sweeps run on that; a delta (h_final - h_sent)
    rides a second ReduceScatter and a correction GEMM patches x_pre1 before
    the fp16 cleanup sweeps, restoring send-last accuracy.
  - Exchange 1 (head) sends final h per-chunk as the last sweep drains,
    overlapped with the th/tm own-half GEMM (held-open PSUM accumulation).
  - Score grid: tanh(h+m) = (th+tm)/(1+th*tm), 1/(1+u) Taylor J=2 (3 terms,
    empirically identical error to J=3) -> MLP grid + biaffine in ONE GEMM.
  - scores emitted as fp16 (halves the output DMA), converted host-side.

Layout: feature/hidden on partitions (chunks of 128), time on free dim.
"""
import numpy as np
import ml_dtypes

import concourse.bass as bass
import concourse.mybir as mybir
import concourse.tile as tile
from concourse import bacc
from concourse.bass import ts, ds
from concourse.bass_utils import run_bass_kernel_spmd
from concourse.masks import make_identity

F32 = mybir.dt.float32
F16 = mybir.dt.float16
F8 = mybir.dt.float8e4
I32 = mybir.dt.int32
AF = mybir.ActivationFunctionType
OP = mybir.AluOpType
DR = mybir.MatmulPerfMode.DoubleRow

N = 768
EW, EP = 300, 64
DIN0 = 384               # 364 padded to 384: word 0:300, pad, pos at 320:384
H = 512
G4 = 4 * H               # 2048
M_MLP = 256
N_PW = 3                 # tm powers 0..2 (Taylor J=2)

N_F8 = 4                 # fp8 DoubleRow sweeps per layer
N_F16 = 2                # fp16 cleanup sweeps per layer
N_ITER = N_F8 + N_F16
N_CORES = 8

GMT = {"i": 0, "f": 1, "g": 2, "o": 3}   # torch gate packing order
MT_ORDER = [GMT[g] * 4 + j for j in range(4) for g in "gifo"]  # j-major


def _rev_view(ap, width):
    """Negative-stride view of a [p, width] AP (reversed along free dim)."""
    return bass.AP(tensor=ap.tensor, offset=ap.offset + (width - 1),
                   ap=[list(ap.ap[0]), [-1, width]])


def _flat_view(ap, rows, cols):
    """[128, a, b] contiguous tile AP -> [128, a*b] view."""
    return bass.AP(tensor=ap.tensor, offset=ap.offset,
                   ap=[list(ap.ap[0]), [1, rows * cols]])


def build_module():
    nc = bacc.Bacc("TRN2", target_bir_lowering=False, debug=False)

    def inp(name, shape, dtype=F32):
        return nc.declare_dram_parameter(name, list(shape), dtype, isOutput=False)

    widx = inp("widx", [N], I32)
    pidx = inp("pidx", [N], I32)
    wemb = inp("wemb", [50000, EW])
    pemb = inp("pemb", [64, EP])
    wih0 = inp("wih0_t", [DIN0, G4], F16)   # per-core: own direction, padded-T
    whh0_16 = inp("whh0_16", [H, G4], F16)
    whh0_8 = inp("whh0_8", [128, 4, G4], F8)
    b0 = inp("b0", [G4])
    wih1 = inp("wih1_t", [2 * H, G4], F16)  # per-core: rows [partner; own]
    whh1_16 = inp("whh1_16", [H, G4], F16)
    whh1_8 = inp("whh1_8", [128, 4, G4], F8)
    b1 = inp("b1", [G4])
    wh_t = inp("wh_t", [2 * H, M_MLP], F16)  # per-core: rows [partner; own]
    wm_t = inp("wm_t", [2 * H, M_MLP], F16)
    bh_in = inp("bh", [M_MLP])
    bm_in = inp("bm", [M_MLP])
    a_t = inp("a_t", [M_MLP + 1, M_MLP + 1], F16)
    wf_in = inp("wf", [M_MLP])
    bf_in = inp("bf", [1])
    sidx = inp("sidx", [128, 4], I32)        # scatter rows: partner slot, per chunk

    scores = nc.declare_dram_parameter("scores", [N, N], F16, isOutput=True)

    cc_in = [nc.dram_tensor(f"cc_in{i}", [2, 4, 128, N], F16) for i in range(2)]
    cc_out = [nc.dram_tensor(f"cc_out{i}", [4, 128, N], F16) for i in range(2)]
    cc_in_d = nc.dram_tensor("cc_in_d", [2, 4, 128, N], F16)
    cc_out_d = nc.dram_tensor("cc_out_d", [4, 128, N], F16)

    PAIRS = [[0, 4], [1, 5], [2, 6], [3, 7]]

    with tile.TileContext(nc) as tc:
        with tc.tile_pool(name="top", bufs=1) as top, \
             tc.tile_pool(name="psum", bufs=4, space="PSUM") as psum:

            # ===== critical-path DMAs first: indices, embeddings, L0 weights
            idxw_sb = top.tile([128, 6], I32, tag="idxw")
            nc.sync.dma_start(out=idxw_sb, in_=widx.rearrange("(a p) -> p a", p=128))
            idxp_sb = top.tile([128, 6], I32, tag="idxp")
            nc.sync.dma_start(out=idxp_sb, in_=pidx.rearrange("(a p) -> p a", p=128))

            ident = top.tile([128, 128], F32)
            make_identity(nc, ident)
            ident16 = top.tile([128, 128], F16)
            nc.vector.tensor_copy(out=ident16, in_=ident)

            wrows = top.tile([128, 6, EW], F32, tag="wrow", name="wrow")
            prows = top.tile([128, 6, EP], F32, tag="prow", name="prow")
            for a in range(6):
                nc.gpsimd.indirect_dma_start(
                    out=wrows[:, a, :], out_offset=None, in_=wemb[:, :],
                    in_offset=bass.IndirectOffsetOnAxis(ap=idxw_sb[:, a:a + 1], axis=0))
            for a in range(6):
                nc.gpsimd.indirect_dma_start(
                    out=prows[:, a, :], out_offset=None, in_=pemb[:, :],
                    in_offset=bass.IndirectOffsetOnAxis(ap=idxp_sb[:, a:a + 1], axis=0))

            wt0 = []
            for kk in range(3):
                wtile = top.tile([128, G4], F16, tag=f"w0_{kk}", name=f"w0_{kk}")
                nc.sync.dma_start(out=wtile, in_=wih0[ds(kk * 128, 128), :])
                wt0.append(wtile)
            u16_0 = top.tile([128, 4, G4], F16, tag="u16_0", name="u16_0")
            for kk in range(4):
                nc.sync.dma_start(out=u16_0[:, kk, :], in_=whh0_16[ds(kk * 128, 128), :])
            u8_0 = top.tile([128, 4, G4], F8, tag="u8_0", name="u8_0")
            nc.sync.dma_start(out=u8_0, in_=whh0_8[:, :, :])
            b_sb = {}
            for lay, bi in ((0, b0), (1, b1)):
                t = top.tile([128, 16], F32, tag=f"bias{lay}", name=f"bias{lay}")
                nc.sync.dma_start(out=t, in_=bi.rearrange("(m p) -> p m", p=128))
                b_sb[lay] = t
            si_sb = top.tile([128, 4], I32, tag="si")
            nc.sync.dma_start(out=si_sb, in_=sidx[:, :])

            # ===== deferred prefetch: L1 + head weights, cc_in zero slots =====
            wt1 = []
            for kk in range(8):
                wtile = top.tile([128, G4], F16, tag=f"w1_{kk}", name=f"w1_{kk}")
                nc.sync.dma_start(out=wtile, in_=wih1[ds(kk * 128, 128), :])
                wt1.append(wtile)
            wf_sb = top.tile([128, 2], F32)
            nc.sync.dma_start(out=wf_sb, in_=wf_in.rearrange("(c p) -> p c", p=128))
            negwf_sb = top.tile([128, 2], F32)
            nc.vector.tensor_scalar_mul(negwf_sb, wf_sb, -1.0)
            bf_sb = top.tile([128, 1], F32)
            nc.sync.dma_start(out=bf_sb, in_=bf_in[:].unsqueeze(0).to_broadcast([128, 1]))
            bh_sb = top.tile([128, 2], F32)
            nc.sync.dma_start(out=bh_sb, in_=bh_in.rearrange("(c p) -> p c", p=128))
            bm_sb = top.tile([128, 2], F32)
            nc.sync.dma_start(out=bm_sb, in_=bm_in.rearrange("(c p) -> p c", p=128))

            zstage = top.tile([128, 4 * N], F16, tag="zstage", name="zstage")
            nc.vector.memset(zstage, 0.0)
            for tgt in (cc_in[0], cc_in[1], cc_in_d):
                for s in range(2):
                    for j in range(4):
                        nc.sync.dma_start(out=tgt[s, j, :, :],
                                          in_=zstage[:, ts(j, N)])
            h_snap = top.tile([128, 4, N], F16, tag="hsnap", name="hsnap")

            own16 = [top.tile([128, 4, N], F16, tag=f"own{l}", name=f"own{l}")
                     for l in range(2)]
            xp16 = [top.tile([128, 4, N], F16, tag=f"xp{l}", name=f"xp{l}")
                    for l in range(2)]

            def fill_t(dst, value, pool, shape=None):
                shape = list(dst.shape) if shape is None else shape
                t = pool.tile(shape, F32, tag="zfill", name="zfill")
                nc.vector.memset(t, value)
                nc.vector.tensor_copy(out=dst, in_=t)

            # ============ LSTM Gauss-Seidel Picard phase ============
            NCH = [(0, 512), (512, 256)]

            def lstm_sweeps(x_pre, u16, u8, bias_tile, out16, chunk_done=None,
                            snap=None, snap_done=None, sweep_hook=None):
                """chunk_done(j): after the final sweep finalizes out16[:,j,:].
                snap/snap_done: snapshot h into `snap` after sweep N_ITER-2
                chunk-by-chunk (stale exchange payload). sweep_hook(k): called
                before emitting sweep k (used to inject the deferred
                delta-correction of x_pre mid-iteration)."""
                with tc.tile_pool(name="phc", bufs=1) as phc:
                    hb16 = phc.tile([128, 4, N + 1], F16, tag="hb16", name="hb16")
                    hb8 = phc.tile([128, 4, N + 1], F8, tag="hb8", name="hb8")
                    nc.vector.memset(hb16, 0.0)
                    nc.vector.memset(hb8, 0.0)
                    it = phc

                    def chain(j, gts, h_dst):
                        """bt -> scan -> tanh -> h write for chunk j."""
                        bt = it.tile([128, N], F16, tag="bt", bufs=2)
                        nc.vector.tensor_tensor(out=bt, in0=gts["i"],
                                                in1=gts["g"], op=OP.mult)
                        ct = it.tile([128, N], F16, tag="ct", bufs=3)
                        nc.vector.tensor_tensor_scan(
                            out=ct, data0=gts["f"], data1=bt, initial=0.0,
                            op0=OP.mult, op1=OP.add)
                        tct = it.tile([128, N], F16, tag="tct", bufs=3)
                        nc.scalar.activation(out=tct, in_=ct, func=AF.Tanh)
                        nc.vector.tensor_tensor(out=h_dst, in0=gts["o"],
                                                in1=tct, op=OP.mult)

                    def act_gates(j, zsrc):
                        gts = {}
                        for g in "gifo":
                            mt = GMT[g] * 4 + j
                            gt = it.tile([128, N], F16, tag=f"g{g}", name=f"g{g}", bufs=4)
                            nc.scalar.activation(
                                out=gt, in_=zsrc[g],
                                func=AF.Tanh if g == "g" else AF.Sigmoid,
                                bias=bias_tile[:, mt:mt + 1], scale=1.0)
                            gts[g] = gt
                        return gts

                    for k in range(N_F8):
                        # fp8 DoubleRow sweeps, GS2 pair-hop order: chunks
                        # {0,1} then {2,3}; within a hop all reads are
                        # pre-hop state so the two chunks' chains overlap.
                        if sweep_hook is not None:
                            sweep_hook(k)
                        last8 = k == N_F8 - 1
                        for grp in ((0, 1), (2, 3)):
                            zps = {}
                            # frontload: seeds + pairs with no fresh dep
                            if not (k == 0 and grp == (0, 1)):
                                for j in grp:
                                    for g in "gifo":
                                        mt = GMT[g] * 4 + j
                                        zp = psum.tile([128, N], F32, tag="zp")
                                        zps[(j, g)] = zp
                                        for (n0, nw) in NCH:
                                            nc.tensor.matmul(
                                                out=zp[:, ds(n0, nw)],
                                                lhsT=ident16[:, :],
                                                rhs=x_pre[:, mt, ds(n0, nw)],
                                                start=True, stop=False)
                                # pair order: the pair updated most recently
                                # goes last (pair 0 fresh when grp=(2,3)).
                                pair_order = [1, 0] if grp == (2, 3) else [0, 1]
                                if k == 0:
                                    pair_order = [0]  # grp (2,3): only pair 0 nonzero
                                for i_a, a in enumerate(pair_order):
                                    is_last = i_a == len(pair_order) - 1
                                    for j in grp:
                                        for g in "gifo":
                                            mt = GMT[g] * 4 + j
                                            for (n0, nw) in NCH:
                                                nc.tensor.matmul(
                                                    out=zps[(j, g)][:, ds(n0, nw)],
                                                    lhsT=u8[:, 2 * a:2 * a + 2, ts(mt, 128)],
                                                    rhs=hb8[:, 2 * a:2 * a + 2, ds(n0, nw)],
                                                    start=False, stop=is_last,
                                                    perf_mode=DR)
                            gts_all = {}
                            for j in grp:
                                if k == 0 and grp == (0, 1):
                                    zsrc = {g: x_pre[:, GMT[g] * 4 + j, 0:N] for g in "gifo"}
                                else:
                                    zsrc = {g: zps[(j, g)][:, 0:N] for g in "gifo"}
                                gts_all[j] = act_gates(j, zsrc)
                            for j in grp:
                                if last8:
                                    chain(j, gts_all[j], hb16[:, j, 1:N + 1])
                                    nc.vector.tensor_copy(out=hb8[:, j, 1:N + 1],
                                                          in_=hb16[:, j, 1:N + 1])
                                else:
                                    # store fp8 directly; hb16 not needed yet
                                    chain(j, gts_all[j], hb8[:, j, 1:N + 1])

                    for k in range(N_F16):
                        if sweep_hook is not None:
                            sweep_hook(N_F8 + k)
                        last = k == N_F16 - 1
                        snapsw = k == N_F16 - 2 and snap is not None
                        for j in range(4):
                            kk_set = [(j + i) % 4 for i in range(4)]
                            zps = {}
                            for g in "gifo":
                                mt = GMT[g] * 4 + j
                                zp = psum.tile([128, N], F32, tag="zp")
                                zps[g] = zp
                                for (n0, nw) in NCH:
                                    nc.tensor.matmul(
                                        out=zp[:, ds(n0, nw)],
                                        lhsT=ident16[:, :],
                                        rhs=x_pre[:, mt, ds(n0, nw)],
                                        start=True, stop=False)
                                for kk in kk_set[:-1]:
                                    for (n0, nw) in NCH:
                                        nc.tensor.matmul(
                                            out=zp[:, ds(n0, nw)],
                                            lhsT=u16[:, kk, ts(mt, 128)],
                                            rhs=hb16[:, kk, ds(n0, nw)],
                                            start=False, stop=False)
                            for g in "gifo":
                                mt = GMT[g] * 4 + j
                                zp = zps[g]
                                kk = kk_set[-1]
                                for (n0, nw) in NCH:
                                    nc.tensor.matmul(
                                        out=zp[:, ds(n0, nw)],
                                        lhsT=u16[:, kk, ts(mt, 128)],
                                        rhs=hb16[:, kk, ds(n0, nw)],
                                        start=False, stop=True)
                            gts = act_gates(j, {g: zps[g][:, 0:N] for g in "gifo"})
                            chain(j, gts, hb16[:, j, 1:N + 1])
                            if snapsw:
                                nc.vector.tensor_copy(out=snap[:, j, :],
                                                      in_=hb16[:, j, 1:N + 1])
                                if snap_done is not None:
                                    snap_done(j)
                            if last:
                                nc.vector.tensor_copy(out=out16[:, j, :],
                                                      in_=hb16[:, j, 1:N + 1])
                                if chunk_done is not None:
                                    chunk_done(j)

            # ===== exchange: scatter own h into partner slot, pair ReduceScatter
            def send_chunk_buf(src_t, j, ci, co):
                flat = ci.rearrange("s j p w -> (s j p) w")
                nc.gpsimd.indirect_dma_start(
                    out=flat,
                    out_offset=bass.IndirectOffsetOnAxis(ap=si_sb[:, j:j + 1], axis=0),
                    in_=src_t[:, j, :], in_offset=None)
                if j == 3:
                    nc.gpsimd.collective_compute(
                        "ReduceScatter", OP.add, replica_groups=PAIRS,
                        ins=[ci[:, :, :, :]], outs=[co[:, :, :]])

            def send_chunk(l, j):
                send_chunk_buf(own16[l], j, cc_in[l], cc_out[l])

            def recv(l, exc):
                raw = exc.tile([128, 4, N], F16, tag="grw", name="grw")
                for j in range(4):
                    nc.sync.dma_start(out=raw[:, j, :], in_=cc_out[l][j, :, :])
                    nc.vector.tensor_copy(out=xp16[l][:, j, :],
                                          in_=_rev_view(raw[:, j, :], N))

            # ============ Phase 0: embeddings ============
            with tc.tile_pool(name="x0t", bufs=1) as x0t:
                x0_T = [x0t.tile([128, N], F16, tag="x0t0", name="x0t0"),
                        x0t.tile([128, N], F16, tag="x0t1", name="x0t1"),
                        x0t.tile([128, N], F16, tag="x0t2", name="x0t2")]
                with tc.tile_pool(name="emb", bufs=2) as embp:
                    fill_t(x0_T[2], 0.0, embp)
                    for a in range(6):
                        for c, (c0, cw) in enumerate([(0, 128), (128, 128), (256, 44)]):
                            tp = psum.tile([128, 128], F32, tag="zp", name="tp")
                            nc.tensor.transpose(tp[:cw, :], wrows[:, a, ds(c0, cw)], ident)
                            if c < 2:
                                nc.vector.tensor_copy(out=x0_T[c][:, ts(a, 128)], in_=tp[:cw, :])
                            else:
                                nc.vector.tensor_copy(out=x0_T[2][0:44, ts(a, 128)], in_=tp[:44, :])
                        tp = psum.tile([128, 128], F32, tag="zp", name="tp")
                        nc.tensor.transpose(tp[:EP, :], prows[:, a, :], ident)
                        nc.vector.tensor_copy(out=x0_T[2][64:128, ts(a, 128)], in_=tp[:EP, :])

                # ============ layer 0 ============
                with tc.tile_pool(name="ph0", bufs=1) as ph0:
                    x_pre0 = ph0.tile([128, 16, N], F16, tag="xpre0")
                    for mt in MT_ORDER:
                        zp = psum.tile([128, N], F32, tag="zp")
                        for kk in range(3):
                            for (n0, nw) in NCH:
                                nc.tensor.matmul(
                                    out=zp[:, ds(n0, nw)],
                                    lhsT=wt0[kk][:, ts(mt, 128)],
                                    rhs=x0_T[kk][:, ds(n0, nw)],
                                    start=(kk == 0), stop=(kk == 2))
                        nc.vector.tensor_copy(out=x_pre0[:, mt, :], in_=zp)
                    def _stale_send0(j):
                        send_chunk_buf(h_snap, j, cc_in[0], cc_out[0])

                    def _delta_send0(j):
                        # delta = final - snapshot (overwrite snapshot)
                        nc.vector.tensor_tensor(
                            out=h_snap[:, j, :], in0=own16[0][:, j, :],
                            in1=h_snap[:, j, :], op=OP.subtract)
                        send_chunk_buf(h_snap, j, cc_in_d, cc_out_d)

                    lstm_sweeps(x_pre0, u16_0, u8_0, b_sb[0], own16[0],
                                snap=h_snap, snap_done=_stale_send0,
                                chunk_done=_delta_send0)

            # ===== exchange 0 overlapped with layer-1 own-half x_pre =====
            with tc.tile_pool(name="ph1", bufs=1) as ph1:
                x_pre1 = ph1.tile([128, 16, N], F16, tag="xpre1")
                u16_1 = ph1.tile([128, 4, G4], F16, tag="u16_1", name="u16_1")
                for kk in range(4):
                    nc.sync.dma_start(out=u16_1[:, kk, :],
                                      in_=whh1_16[ds(kk * 128, 128), :])
                u8_1 = ph1.tile([128, 4, G4], F8, tag="u8_1", name="u8_1")
                nc.sync.dma_start(out=u8_1, in_=whh1_8[:, :, :])
                with tc.tile_pool(name="exc0", bufs=1) as exc0:
                    # pass A: own-direction half (rows 512:1024 = wt1[4:8])
                    for mt in MT_ORDER:
                        zp = psum.tile([128, N], F32, tag="zp")
                        for i_kk, kk in enumerate(range(4)):
                            for (n0, nw) in NCH:
                                nc.tensor.matmul(
                                    out=zp[:, ds(n0, nw)],
                                    lhsT=wt1[4 + kk][:, ts(mt, 128)],
                                    rhs=own16[0][:, kk, ds(n0, nw)],
                                    start=(i_kk == 0), stop=(i_kk == 3))
                        nc.vector.tensor_copy(out=x_pre1[:, mt, :], in_=zp)
                    recv(0, exc0)
                    # pass B: partner half accumulated on top
                    for mt in MT_ORDER:
                        zp = psum.tile([128, N], F32, tag="zp")
                        for i_kk, kk in enumerate(range(4)):
                            for (n0, nw) in NCH:
                                nc.tensor.matmul(
                                    out=zp[:, ds(n0, nw)],
                                    lhsT=wt1[kk][:, ts(mt, 128)],
                                    rhs=xp16[0][:, kk, ds(n0, nw)],
                                    start=(i_kk == 0), stop=(i_kk == 3))
                        nc.vector.tensor_tensor(out=x_pre1[:, mt, :], in0=x_pre1[:, mt, :],
                                                in1=zp, op=OP.add)

                # ============ layer 1 ============
                def _hook1(k):
                    if k != 3:
                        return
                    # delta correction: x_pre1 += Wih1_partner @ rev(delta)
                    # (h_snap is dead after the delta send -- reuse as recv buf)
                    for j in range(4):
                        nc.sync.dma_start(out=h_snap[:, j, :], in_=cc_out_d[j, :, :])
                        nc.vector.tensor_copy(out=xp16[0][:, j, :],
                                              in_=_rev_view(h_snap[:, j, :], N))
                    for mt in MT_ORDER:
                        zp = psum.tile([128, N], F32, tag="zp")
                        for i_kk, kk in enumerate(range(4)):
                            for (n0, nw) in NCH:
                                nc.tensor.matmul(
                                    out=zp[:, ds(n0, nw)],
                                    lhsT=wt1[kk][:, ts(mt, 128)],
                                    rhs=xp16[0][:, kk, ds(n0, nw)],
                                    start=(i_kk == 0), stop=(i_kk == 3))
                        nc.vector.tensor_tensor(out=x_pre1[:, mt, :],
                                                in0=x_pre1[:, mt, :],
                                                in1=zp, op=OP.add)

                lstm_sweeps(x_pre1, u16_1, u8_1, b_sb[1], own16[1],
                            chunk_done=lambda j: send_chunk(1, j),
                            sweep_hook=_hook1)

            # ===== exchange 1 + head (th/tm own-half overlapped in PSUM) =====
            with tc.tile_pool(name="head", bufs=1) as hd:
                wtiles = {}
                for wi, w_dram in enumerate((wh_t, wm_t)):
                    for kk in (4, 5, 6, 7, 0, 1, 2, 3):
                        wr = hd.tile([128, M_MLP], F16, tag=f"hw{wi}_{kk}",
                                     name=f"hw{wi}_{kk}")
                        nc.sync.dma_start(out=wr, in_=w_dram[ds(kk * 128, 128), :])
                        wtiles[(wi, kk)] = wr
                at_tiles = []
                for kk, pk in ((0, 128), (1, 128), (2, 1)):
                    wr = hd.tile([128, M_MLP + 1], F16, tag=f"at_r{kk}", name=f"at_r{kk}")
                    nc.sync.dma_start(out=wr[:pk, :], in_=a_t[ds(kk * 128, pk), :])
                    at_tiles.append(wr)
                th_r = [hd.tile([128, N], F16, tag=f"thr{c}", name=f"thr{c}") for c in range(2)]
                tm_r = [hd.tile([128, N], F16, tag=f"tmr{c}", name=f"tmr{c}") for c in range(2)]
                with tc.tile_pool(name="exc1", bufs=1) as exc1:
                    # pass A: own half (rows 512:1024) into held-open PSUM
                    zps = {}
                    for wi in range(2):
                        for mt in range(2):
                            zp = psum.tile([128, N], F32, tag="zp",
                                           name=f"zph{wi}{mt}")
                            zps[(wi, mt)] = zp
                            for i_kk, kk in enumerate(range(4)):
                                for (n0, nw) in NCH:
                                    nc.tensor.matmul(out=zp[:, ds(n0, nw)],
                                                     lhsT=wtiles[(wi, 4 + kk)][:, ts(mt, 128)],
                                                     rhs=own16[1][:, kk, ds(n0, nw)],
                                                     start=(i_kk == 0), stop=False)
                    recv(1, exc1)
                    # pass B: partner half, close accumulation, tanh
                    for wi, (bias_t, dst) in enumerate(((bh_sb, th_r), (bm_sb, tm_r))):
                        for mt in range(2):
                            zp = zps[(wi, mt)]
                            for i_kk, kk in enumerate(range(4)):
                                for (n0, nw) in NCH:
                                    nc.tensor.matmul(out=zp[:, ds(n0, nw)],
                                                     lhsT=wtiles[(wi, kk)][:, ts(mt, 128)],
                                                     rhs=xp16[1][:, kk, ds(n0, nw)],
                                                     start=False, stop=(i_kk == 3))
                            nc.scalar.activation(out=dst[mt], in_=zp, func=AF.Tanh,
                                                 bias=bias_t[:, mt:mt + 1], scale=1.0)

                ones_row = hd.tile([1, N], F16, tag="ones1")
                with tc.tile_pool(name="zf2", bufs=1) as zf2:
                    fill_t(ones_row, 1.0, zf2, shape=[1, N])

                # Q_att = A @ mb_^T
                q_att = [hd.tile([128, N], F16, tag="qa0", name="qa0"),
                         hd.tile([128, N], F16, tag="qa1", name="qa1"),
                         hd.tile([1, N], F16, tag="qa2", name="qa2")]
                rhs_mb = [(tm_r[0], 128), (tm_r[1], 128), (ones_row, 1)]
                for mt, mw in ((0, 128), (1, 128), (2, 1)):
                    zp = psum.tile([128, N], F32, tag="zp")
                    for kk, (rt, pk) in enumerate(rhs_mb):
                        for (n0, nw) in NCH:
                            nc.tensor.matmul(out=zp[:mw, ds(n0, nw)],
                                             lhsT=at_tiles[kk][:pk, ds(mt * 128, mw)],
                                             rhs=rt[:pk, ds(n0, nw)],
                                             start=(kk == 0), stop=(kk == 2))
                    nc.vector.tensor_copy(out=q_att[mt][:mw, :], in_=zp[:mw, :])

                # P/Q Taylor blocks (all fp16: 2x DVE)
                p_mlp = [[hd.tile([128, N], F16, tag=f"pm{p}_{c}", name=f"pm{p}_{c}")
                          for c in range(2)] for p in range(N_PW)]
                q_mlp = [[hd.tile([128, N], F16, tag=f"qm{p}_{c}", name=f"qm{p}_{c}")
                          for c in range(2)] for p in range(N_PW)]
                for c in range(2):
                    wfc = wf_sb[:, c:c + 1]
                    nwfc = negwf_sb[:, c:c + 1]
                    th2 = hd.tile([128, N], F16, tag="th2")
                    nc.vector.tensor_tensor(out=th2, in0=th_r[c], in1=th_r[c], op=OP.mult)
                    negw1 = hd.tile([128, N], F16, tag="negw1")
                    nc.vector.tensor_scalar(out=negw1, in0=th2, scalar1=wfc, scalar2=nwfc,
                                            op0=OP.mult, op1=OP.add)
                    nc.vector.tensor_scalar_mul(p_mlp[0][c], th_r[c], wfc)
                    nc.vector.tensor_scalar(out=p_mlp[1][c], in0=th2, scalar1=nwfc, scalar2=wfc,
                                            op0=OP.mult, op1=OP.add)
                    nc.vector.tensor_tensor(out=p_mlp[2][c], in0=th_r[c], in1=negw1, op=OP.mult)
                    one_t = hd.tile([128, N], F16, tag="one_t")
                    nc.vector.memset(one_t, 1.0)
                    nc.vector.tensor_copy(out=q_mlp[0][c], in_=one_t)
                    nc.vector.tensor_copy(out=q_mlp[1][c], in_=tm_r[c])
                    nc.vector.tensor_tensor(out=q_mlp[2][c], in0=tm_r[c], in1=tm_r[c], op=OP.mult)

                kblocks = [(th_r[0], q_att[0], 128), (th_r[1], q_att[1], 128),
                           (ones_row, q_att[2], 1)]
                for p in range(N_PW):
                    for c in range(2):
                        kblocks.append((p_mlp[p][c], q_mlp[p][c], 128))
                nkb = len(kblocks)
                for xt in range(6):
                    zp = psum.tile([128, N], F32, tag="zp")
                    for kb, (pt, qt, pk) in enumerate(kblocks):
                        for (n0, nw) in NCH:
                            nc.tensor.matmul(out=zp[:, ds(n0, nw)],
                                             lhsT=pt[:pk, ts(xt, 128)],
                                             rhs=qt[:pk, ds(n0, nw)],
                                             start=(kb == 0), stop=(kb == nkb - 1))
                    srow = hd.tile([128, N], F16, tag="srow", bufs=2)
                    nc.scalar.activation(out=srow, in_=zp, func=AF.Identity,
                                         bias=bf_sb, scale=1.0)
                    nc.sync.dma_start(out=scores[ts(xt, 128), :], in_=srow)

    nc.finalize()
    return nc


_NC_CACHE = {}


def _get_module():
    key = (N_F8, N_F16, N_PW)
    if key not in _NC_CACHE:
        _NC_CACHE[key] = build_module()
    return _NC_CACHE[key]


def _pad_wih0(wt):
    """[364, G4] -> [384, G4]: word rows 0:300, zeros, pos rows at 320:384."""
    pad = np.zeros((DIN0, wt.shape[1]), np.float32)
    pad[0:300] = wt[0:300]
    pad[320:384] = wt[300:364]
    return pad


def _to_f8(w):
    return np.ascontiguousarray(w.astype(ml_dtypes.float8_e4m3))


def _whh8(whh_t):
    """[512, 2048] f32 (Whh.T) -> [128, 4, 2048] fp8 (k-tile layout)."""
    w = np.asarray(whh_t, np.float32).reshape(4, 128, G4).transpose(1, 0, 2)
    return _to_f8(w)


def _prep_inputs_core(inputs, core):
    f32, f16 = np.float32, np.float16
    is_f = core < 4
    d = "f" if is_f else "b"
    widx = np.asarray(inputs["word_idx"]).reshape(-1).astype(np.int32)
    pidx = np.asarray(inputs["pos_idx"]).reshape(-1).astype(np.int32)
    if not is_f:
        widx = widx[::-1]
        pidx = pidx[::-1]
    wih1 = np.asarray(inputs[f"Wih1{d}"]).T.astype(f32)     # [1024, 2048]
    wh = np.asarray(inputs["Wh"]).T.astype(f32)             # [1024, 256]
    wm = np.asarray(inputs["Wm"]).T.astype(f32)
    if is_f:
        # program's x order is [partner(=b); own(=f)] -> permute rows
        wih1 = np.concatenate([wih1[512:1024], wih1[0:512]], 0)
        wh = np.concatenate([wh[512:1024], wh[0:512]], 0)
        wm = np.concatenate([wm[512:1024], wm[0:512]], 0)
    whh0_t = np.asarray(inputs[f"Whh0{d}"]).T.astype(f32)   # [512, 2048]
    whh1_t = np.asarray(inputs[f"Whh1{d}"]).T.astype(f32)
    # scatter own h into the slot of the PARTNER's rank (fwd rank 0, bwd 1)
    rank = 0 if is_f else 1
    sidx_arr = np.stack(
        [((1 - rank) * 4 + j) * 128 + np.arange(128) for j in range(4)],
        axis=1).astype(np.int32)
    im = {
        "widx": np.ascontiguousarray(widx),
        "pidx": np.ascontiguousarray(pidx),
        "wemb": np.ascontiguousarray(inputs["word_emb"], dtype=f32),
        "pemb": np.ascontiguousarray(inputs["pos_emb"], dtype=f32),
        "wih0_t": np.ascontiguousarray(
            _pad_wih0(np.asarray(inputs[f"Wih0{d}"]).T.astype(f32)).astype(f16)),
        "whh0_16": np.ascontiguousarray(whh0_t.astype(f16)),
        "whh0_8": _whh8(whh0_t),
        "b0": np.ascontiguousarray(inputs[f"b0{d}"], dtype=f32),
        "wih1_t": np.ascontiguousarray(wih1.astype(f16)),
        "whh1_16": np.ascontiguousarray(whh1_t.astype(f16)),
        "whh1_8": _whh8(whh1_t),
        "b1": np.ascontiguousarray(inputs[f"b1{d}"], dtype=f32),
        "wh_t": np.ascontiguousarray(wh.astype(f16)),
        "wm_t": np.ascontiguousarray(wm.astype(f16)),
        "bh": np.ascontiguousarray(inputs["bh"], dtype=f32),
        "bm": np.ascontiguousarray(inputs["bm"], dtype=f32),
        "a_t": np.ascontiguousarray(np.asarray(inputs["A"])[0].T.astype(f16)),
        "wf": np.ascontiguousarray(np.asarray(inputs["Wf"]).reshape(-1), dtype=f32),
        "bf": np.ascontiguousarray(np.asarray(inputs["bf"]).reshape(-1), dtype=f32),
        "sidx": sidx_arr,
    }
    return im


_RUNNER_CACHE = {}


def _get_runner():
    """Cached jitted 8-core runner (mirrors bass2jax.run_bass_via_pjrt's
    multi-core path, but reuses the compiled executable across calls)."""
    key = (N_F8, N_F16, N_PW)
    if key in _RUNNER_CACHE:
        return _RUNNER_CACHE[key]
    import jax
    from jax.sharding import Mesh, PartitionSpec
    from jax.experimental.shard_map import shard_map
    from concourse.bass2jax import (_bass_exec_p, install_neuronx_cc_hook,
                                    partition_id_tensor)
    nc = _get_module()
    install_neuronx_cc_hook()
    partition_name = nc.partition_id_tensor.name if nc.partition_id_tensor else None
    in_names, out_names, out_avals, zero_shapes = [], [], [], []
    for alloc in nc.m.functions[0].allocations:
        if not isinstance(alloc, mybir.MemoryLocationSet):
            continue
        name = alloc.memorylocations[0].name
        if alloc.kind == "ExternalInput":
            if name != partition_name:
                in_names.append(name)
        elif alloc.kind == "ExternalOutput":
            shape = tuple(alloc.tensor_shape)
            dtype = mybir.dt.np(alloc.dtype)
            out_avals.append(jax.core.ShapedArray(shape, dtype))
            out_names.append(name)
            zero_shapes.append((shape, dtype))
    n_params, n_outs = len(in_names), len(out_names)
    full_in_names = list(in_names) + list(out_names)
    if partition_name is not None:
        full_in_names.append(partition_name)
    donate = tuple(range(n_params, n_params + n_outs))

    def _body(*args):
        operands = list(args)
        if partition_name is not None:
            operands.append(partition_id_tensor())
        outs = _bass_exec_p.bind(
            *operands, out_avals=tuple(out_avals), in_names=tuple(full_in_names),
            out_names=tuple(out_names), lowering_input_output_aliases=(),
            sim_require_finite=True, sim_require_nnan=True, nc=nc)
        return tuple(outs)

    devices = jax.devices()[:N_CORES]
    mesh = Mesh(np.asarray(devices), ("core",))
    sharded = jax.jit(
        shard_map(_body, mesh=mesh,
                  in_specs=(PartitionSpec("core"),) * (n_params + n_outs),
                  out_specs=(PartitionSpec("core"),) * n_outs,
                  check_rep=False),
        donate_argnums=donate, keep_unused=True)

    def run(ims):
        concat_in = [np.concatenate([np.asarray(ims[c][nm]) for c in range(N_CORES)], 0)
                     for nm in in_names]
        concat_zeros = [np.zeros((N_CORES * sh[0], *sh[1:]), dt)
                        for sh, dt in zero_shapes]
        out_arrs = sharded(*concat_in, *concat_zeros)
        return [{nm: np.asarray(out_arrs[i]).reshape(N_CORES, *out_avals[i].shape)[c]
                 for i, nm in enumerate(out_names)} for c in range(N_CORES)]

    _RUNNER_CACHE[key] = run
    return run


def kernel(**inputs) -> np.ndarray:
    inputs = {k: np.asarray(v) for k, v in inputs.items()}
    run = _get_runner()
    ims = [_prep_inputs_core(inputs, c) for c in range(N_CORES)]
    results = run(ims)
    out = np.asarray(results[0]["scores"], dtype=np.float32)
    return np.ascontiguousarray(out.reshape(1, N, N))


def run_debug(inputs, cores=(0,)):
    nc = _get_module()
    inputs = {k: np.asarray(v) for k, v in inputs.items()}
    ims = [_prep_inputs_core(inputs, c) for c in range(N_CORES)]
    res = run_bass_kernel_spmd(nc, ims, core_ids=list(range(N_CORES)))
    return [res.results[c] for c in cores]


# revision 24
# speedup vs baseline: 1.0040x; 1.0040x over previous
"""Trainium2 Bass kernel for nn_DependencyParserCombinedAttention.

Model: embeddings -> 2-layer BiLSTM (H=512) -> biaffine attention + MLP
score grid [1, 768, 768].

Implementation (SPMD over 8 NeuronCores):
  - Direction split: cores 0-3 forward, 4-7 backward (time-reversed inputs);
    4-way redundancy within a direction. Exchange between directions via a
    pair-group ReduceScatter (out = exactly the partner payload, half the
    bytes of an AllGather) with the self slot zeroed and the own hidden
    scattered into the partner slot by an indirect DMA.
  - LSTM recurrence via Picard sweeps. First N_F8 sweeps run the recurrent
    (Whh) matmuls in fp8e4 DoubleRow perf mode (K=256 per matmul at 0.5
    cyc/col -> 4x fewer PE cycles than fp16) in GS2 pair-hop order (chunks
    {0,1} then {2,3}; within a hop both chunks' ACT/DVE chains overlap); the
    last N_F16 sweeps run fp16 in GS4 order. Early fp8 sweeps store h
    directly as fp8 (shorter producer chain); the transition sweep stores
    fp16+fp8. All input GEMMs and the exchange payload stay fp16.
  - Exchange 0 (layer 0 -> 1) is latency-hidden: h after sweep N-2 is sent
    under the final sweep (ReduceScatter 1), x_pre1's partner half is built
    from it and the first L1 sweeps run on that; a delta (h_final - h_sent)
    rides a second ReduceScatter and a correction GEMM patches x_pre1 before
    the fp16 cleanup sweeps, restoring send-last accuracy.
  - Exchange 1 (head) sends final h per-chunk as the last sweep drains,
    overlapped with the th/tm own-half GEMM (held-open PSUM accumulation).
  - Score grid: tanh(h+m) = (th+tm)/(1+th*tm), 1/(1+u) Taylor J=2 (3 terms,
    empirically identical error to J=3) -> MLP grid + biaffine in ONE GEMM.
  - scores emitted as fp16 (halves the output DMA), converted host-side.

Layout: feature/hidden on partitions (chunks of 128), time on free dim.
"""
import numpy as np
import ml_dtypes

import concourse.bass as bass
import concourse.mybir as mybir
import concourse.tile as tile
from concourse import bacc
from concourse.bass import ts, ds
from concourse.bass_utils import run_bass_kernel_spmd
from concourse.masks import make_identity

F32 = mybir.dt.float32
F16 = mybir.dt.float16
F8 = mybir.dt.float8e4
I32 = mybir.dt.int32
AF = mybir.ActivationFunctionType
OP = mybir.AluOpType
DR = mybir.MatmulPerfMode.DoubleRow

N = 768
EW, EP = 300, 64
DIN0 = 384               # 364 padded to 384: word 0:300, pad, pos at 320:384
H = 512
G4 = 4 * H               # 2048
M_MLP = 256
N_PW = 3                 # tm powers 0..2 (Taylor J=2)

N_F8 = 4                 # fp8 DoubleRow sweeps per layer
N_F16 = 2                # fp16 cleanup sweeps per layer
N_ITER = N_F8 + N_F16
N_CORES = 8

GMT = {"i": 0, "f": 1, "g": 2, "o": 3}   # torch gate packing order
MT_ORDER = [GMT[g] * 4 + j for j in range(4) for g in "gifo"]  # j-major


def _rev_view(ap, width):
    """Negative-stride view of a [p, width] AP (reversed along free dim)."""
    return bass.AP(tensor=ap.tensor, offset=ap.offset + (width - 1),
                   ap=[list(ap.ap[0]), [-1, width]])


def _flat_view(ap, rows, cols):
    """[128, a, b] contiguous tile AP -> [128, a*b] view."""
    return bass.AP(tensor=ap.tensor, offset=ap.offset,
                   ap=[list(ap.ap[0]), [1, rows * cols]])


def build_module():
    nc = bacc.Bacc("TRN2", target_bir_lowering=False, debug=False)

    def inp(name, shape, dtype=F32):
        return nc.declare_dram_parameter(name, list(shape), dtype, isOutput=False)

    widx = inp("widx", [N], I32)
    pidx = inp("pidx", [N], I32)
    wemb = inp("wemb", [50000, EW])
    pemb = inp("pemb", [64, EP])
    wih0 = inp("wih0_t", [DIN0, G4], F16)   # per-core: own direction, padded-T
    whh0_16 = inp("whh0_16", [H, G4], F16)
    whh0_8 = inp("whh0_8", [128, 4, G4], F8)
    b0 = inp("b0", [G4])
    wih1 = inp("wih1_t", [2 * H, G4], F16)  # per-core: rows [partner; own]
    whh1_16 = inp("whh1_16", [H, G4], F16)
    whh1_8 = inp("whh1_8", [128, 4, G4], F8)
    b1 = inp("b1", [G4])
    wh_t = inp("wh_t", [2 * H, M_MLP], F16)  # per-core: rows [partner; own]
    wm_t = inp("wm_t", [2 * H, M_MLP], F16)
    bh_in = inp("bh", [M_MLP])
    bm_in = inp("bm", [M_MLP])
    a_t = inp("a_t", [M_MLP + 1, M_MLP + 1], F16)
    wf_in = inp("wf", [M_MLP])
    bf_in = inp("bf", [1])
    sidx = inp("sidx", [128, 4], I32)        # scatter rows: partner slot, per chunk

    scores = nc.declare_dram_parameter("scores", [N, N], F16, isOutput=True)

    cc_in = [nc.dram_tensor(f"cc_in{i}", [2, 4, 128, N], F16) for i in range(2)]
    cc_out = [nc.dram_tensor(f"cc_out{i}", [4, 128, N], F16) for i in range(2)]
    cc_in_d = nc.dram_tensor("cc_in_d", [2, 4, 128, N], F16)
    cc_out_d = nc.dram_tensor("cc_out_d", [4, 128, N], F16)

    PAIRS = [[0, 4], [1, 5], [2, 6], [3, 7]]

    with tile.TileContext(nc) as tc:
        with tc.tile_pool(name="top", bufs=1) as top, \
             tc.tile_pool(name="psum", bufs=4, space="PSUM") as psum:

            # ===== critical-path DMAs first: indices, embeddings, L0 weights
            idxw_sb = top.tile([128, 6], I32, tag="idxw")
            nc.sync.dma_start(out=idxw_sb, in_=widx.rearrange("(a p) -> p a", p=128))
            idxp_sb = top.tile([128, 6], I32, tag="idxp")
            nc.sync.dma_start(out=idxp_sb, in_=pidx.rearrange("(a p) -> p a", p=128))

            ident = top.tile([128, 128], F32)
            make_identity(nc, ident)
            ident16 = top.tile([128, 128], F16)
            nc.vector.tensor_copy(out=ident16, in_=ident)

            wrows = top.tile([128, 6, EW], F32, tag="wrow", name="wrow")
            prows = top.tile([128, 6, EP], F32, tag="prow", name="prow")
            for a in range(6):
                nc.gpsimd.indirect_dma_start(
                    out=wrows[:, a, :], out_offset=None, in_=wemb[:, :],
                    in_offset=bass.IndirectOffsetOnAxis(ap=idxw_sb[:, a:a + 1], axis=0))
            for a in range(6):
                nc.gpsimd.indirect_dma_start(
                    out=prows[:, a, :], out_offset=None, in_=pemb[:, :],
                    in_offset=bass.IndirectOffsetOnAxis(ap=idxp_sb[:, a:a + 1], axis=0))

            wt0 = []
            for kk in range(3):
                wtile = top.tile([128, G4], F16, tag=f"w0_{kk}", name=f"w0_{kk}")
                nc.sync.dma_start(out=wtile, in_=wih0[ds(kk * 128, 128), :])
                wt0.append(wtile)
            u16_0 = top.tile([128, 4, G4], F16, tag="u16_0", name="u16_0")
            for kk in range(4):
                nc.sync.dma_start(out=u16_0[:, kk, :], in_=whh0_16[ds(kk * 128, 128), :])
            u8_0 = top.tile([128, 4, G4], F8, tag="u8_0", name="u8_0")
            nc.sync.dma_start(out=u8_0, in_=whh0_8[:, :, :])
            b_sb = {}
            for lay, bi in ((0, b0), (1, b1)):
                t = top.tile([128, 16], F32, tag=f"bias{lay}", name=f"bias{lay}")
                nc.sync.dma_start(out=t, in_=bi.rearrange("(m p) -> p m", p=128))
                b_sb[lay] = t
            si_sb = top.tile([128, 4], I32, tag="si")
            nc.sync.dma_start(out=si_sb, in_=sidx[:, :])

            # ===== deferred prefetch: L1 + head weights, cc_in zero slots =====
            wt1 = []
            for kk in range(8):
                wtile = top.tile([128, G4], F16, tag=f"w1_{kk}", name=f"w1_{kk}")
                nc.sync.dma_start(out=wtile, in_=wih1[ds(kk * 128, 128), :])
                wt1.append(wtile)
            wf_sb = top.tile([128, 2], F32)
            nc.sync.dma_start(out=wf_sb, in_=wf_in.rearrange("(c p) -> p c", p=128))
            negwf_sb = top.tile([128, 2], F32)
            nc.vector.tensor_scalar_mul(negwf_sb, wf_sb, -1.0)
            bf_sb = top.tile([128, 1], F32)
            nc.sync.dma_start(out=bf_sb, in_=bf_in[:].unsqueeze(0).to_broadcast([128, 1]))
            bh_sb = top.tile([128, 2], F32)
            nc.sync.dma_start(out=bh_sb, in_=bh_in.rearrange("(c p) -> p c", p=128))
            bm_sb = top.tile([128, 2], F32)
            nc.sync.dma_start(out=bm_sb, in_=bm_in.rearrange("(c p) -> p c", p=128))

            zstage = top.tile([128, 4 * N], F16, tag="zstage", name="zstage")
            nc.vector.memset(zstage, 0.0)
            for tgt in (cc_in[0], cc_in[1], cc_in_d):
                for s in range(2):
                    for j in range(4):
                        nc.sync.dma_start(out=tgt[s, j, :, :],
                                          in_=zstage[:, ts(j, N)])
            h_snap = top.tile([128, 4, N], F16, tag="hsnap", name="hsnap")

            own16 = [top.tile([128, 4, N], F16, tag=f"own{l}", name=f"own{l}")
                     for l in range(2)]
            xp16 = [top.tile([128, 4, N], F16, tag=f"xp{l}", name=f"xp{l}")
                    for l in range(2)]

            def fill_t(dst, value, pool, shape=None):
                shape = list(dst.shape) if shape is None else shape
                t = pool.tile(shape, F32, tag="zfill", name="zfill")
                nc.vector.memset(t, value)
                nc.vector.tensor_copy(out=dst, in_=t)

            # ============ LSTM Gauss-Seidel Picard phase ============
            NCH = [(0, 512), (512, 256)]

            def lstm_sweeps(x_pre, u16, u8, bias_tile, out16, chunk_done=None,
                            snap=None, snap_done=None, sweep_hook=None):
                """chunk_done(j): after the final sweep finalizes out16[:,j,:].
                snap/snap_done: snapshot h into `snap` after sweep N_ITER-2
                chunk-by-chunk (stale exchange payload). sweep_hook(k): called
                before emitting sweep k (used to inject the deferred
                delta-correction of x_pre mid-iteration)."""
                with tc.tile_pool(name="phc", bufs=1) as phc:
                    hb16 = phc.tile([128, 4, N + 1], F16, tag="hb16", name="hb16")
                    hb8 = phc.tile([128, 4, N + 1], F8, tag="hb8", name="hb8")
                    nc.vector.memset(hb16, 0.0)
                    nc.vector.memset(hb8, 0.0)
                    it = phc

                    def chain(j, gts, h_dst):
                        """bt -> scan -> tanh -> h write for chunk j."""
                        bt = it.tile([128, N], F16, tag="bt", bufs=2)
                        nc.vector.tensor_tensor(out=bt, in0=gts["i"],
                                                in1=gts["g"], op=OP.mult)
                        ct = it.tile([128, N], F16, tag="ct", bufs=3)
                        nc.vector.tensor_tensor_scan(
                            out=ct, data0=gts["f"], data1=bt, initial=0.0,
                            op0=OP.mult, op1=OP.add)
                        tct = it.tile([128, N], F16, tag="tct", bufs=3)
                        nc.scalar.activation(out=tct, in_=ct, func=AF.Tanh)
                        nc.vector.tensor_tensor(out=h_dst, in0=gts["o"],
                                                in1=tct, op=OP.mult)

                    def act_gates(j, zsrc):
                        gts = {}
                        for g in "gifo":
                            mt = GMT[g] * 4 + j
                            gt = it.tile([128, N], F16, tag=f"g{g}", name=f"g{g}", bufs=4)
                            nc.scalar.activation(
                                out=gt, in_=zsrc[g],
                                func=AF.Tanh if g == "g" else AF.Sigmoid,
                                bias=bias_tile[:, mt:mt + 1], scale=1.0)
                            gts[g] = gt
                        return gts

                    for k in range(N_F8):
                        # fp8 DoubleRow sweeps, GS2 pair-hop order: chunks
                        # {0,1} then {2,3}; within a hop all reads are
                        # pre-hop state so the two chunks' chains overlap.
                        if sweep_hook is not None:
                            sweep_hook(k)
                        last8 = k == N_F8 - 1
                        for grp in ((0, 1), (2, 3)):
                            zps = {}
                            # frontload: seeds + pairs with no fresh dep
                            if not (k == 0 and grp == (0, 1)):
                                for j in grp:
                                    for g in "gifo":
                                        mt = GMT[g] * 4 + j
                                        zp = psum.tile([128, N], F32, tag="zp")
                                        zps[(j, g)] = zp
                                        for (n0, nw) in NCH:
                                            nc.tensor.matmul(
                                                out=zp[:, ds(n0, nw)],
                                                lhsT=ident16[:, :],
                                                rhs=x_pre[:, mt, ds(n0, nw)],
                                                start=True, stop=False)
                                # pair order: the pair updated most recently
                                # goes last (pair 0 fresh when grp=(2,3)).
                                pair_order = [1, 0] if grp == (2, 3) else [0, 1]
                                if k == 0:
                                    pair_order = [0]  # grp (2,3): only pair 0 nonzero
                                for i_a, a in enumerate(pair_order):
                                    is_last = i_a == len(pair_order) - 1
                                    for j in grp:
                                        for g in "gifo":
                                            mt = GMT[g] * 4 + j
                                            for (n0, nw) in NCH:
                                                nc.tensor.matmul(
                                                    out=zps[(j, g)][:, ds(n0, nw)],
                                                    lhsT=u8[:, 2 * a:2 * a + 2, ts(mt, 128)],
                                                    rhs=hb8[:, 2 * a:2 * a + 2, ds(n0, nw)],
                                                    start=False, stop=is_last,
                                                    perf_mode=DR)
                            gts_all = {}
                            for j in grp:
                                if k == 0 and grp == (0, 1):
                                    zsrc = {g: x_pre[:, GMT[g] * 4 + j, 0:N] for g in "gifo"}
                                else:
                                    zsrc = {g: zps[(j, g)][:, 0:N] for g in "gifo"}
                                gts_all[j] = act_gates(j, zsrc)
                            for j in grp:
                                if last8:
                                    chain(j, gts_all[j], hb16[:, j, 1:N + 1])
                                    nc.vector.tensor_copy(out=hb8[:, j, 1:N + 1],
                                                          in_=hb16[:, j, 1:N + 1])
                                    if snap is not None:
                                        nc.vector.tensor_copy(
                                            out=snap[:, j, :],
                                            in_=hb16[:, j, 1:N + 1])
                                        if snap_done is not None:
                                            snap_done(j)
                                else:
                                    # store fp8 directly; hb16 not needed yet
                                    chain(j, gts_all[j], hb8[:, j, 1:N + 1])

                    for k in range(N_F16):
                        if sweep_hook is not None:
                            sweep_hook(N_F8 + k)
                        last = k == N_F16 - 1
                        snapsw = False
                        for j in range(4):
                            kk_set = [(j + i) % 4 for i in range(4)]
                            zps = {}
                            for g in "gifo":
                                mt = GMT[g] * 4 + j
                                zp = psum.tile([128, N], F32, tag="zp")
                                zps[g] = zp
                                for (n0, nw) in NCH:
                                    nc.tensor.matmul(
                                        out=zp[:, ds(n0, nw)],
                                        lhsT=ident16[:, :],
                                        rhs=x_pre[:, mt, ds(n0, nw)],
                                        start=True, stop=False)
                                for kk in kk_set[:-1]:
                                    for (n0, nw) in NCH:
                                        nc.tensor.matmul(
                                            out=zp[:, ds(n0, nw)],
                                            lhsT=u16[:, kk, ts(mt, 128)],
                                            rhs=hb16[:, kk, ds(n0, nw)],
                                            start=False, stop=False)
                            for g in "gifo":
                                mt = GMT[g] * 4 + j
                                zp = zps[g]
                                kk = kk_set[-1]
                                for (n0, nw) in NCH:
                                    nc.tensor.matmul(
                                        out=zp[:, ds(n0, nw)],
                                        lhsT=u16[:, kk, ts(mt, 128)],
                                        rhs=hb16[:, kk, ds(n0, nw)],
                                        start=False, stop=True)
                            gts = act_gates(j, {g: zps[g][:, 0:N] for g in "gifo"})
                            chain(j, gts, hb16[:, j, 1:N + 1])
                            if snapsw:
                                nc.vector.tensor_copy(out=snap[:, j, :],
                                                      in_=hb16[:, j, 1:N + 1])
                                if snap_done is not None:
                                    snap_done(j)
                            if last:
                                nc.vector.tensor_copy(out=out16[:, j, :],
                                                      in_=hb16[:, j, 1:N + 1])
                                if chunk_done is not None:
                                    chunk_done(j)

            # ===== exchange: scatter own h into partner slot, pair ReduceScatter
            def send_chunk_buf(src_t, j, ci, co):
                flat = ci.rearrange("s j p w -> (s j p) w")
                nc.gpsimd.indirect_dma_start(
                    out=flat,
                    out_offset=bass.IndirectOffsetOnAxis(ap=si_sb[:, j:j + 1], axis=0),
                    in_=src_t[:, j, :], in_offset=None)
                if j == 3:
                    nc.gpsimd.collective_compute(
                        "ReduceScatter", OP.add, replica_groups=PAIRS,
                        ins=[ci[:, :, :, :]], outs=[co[:, :, :]])

            def send_chunk(l, j):
                send_chunk_buf(own16[l], j, cc_in[l], cc_out[l])

            def recv(l, exc):
                raw = exc.tile([128, 4, N], F16, tag="grw", name="grw")
                for j in range(4):
                    nc.sync.dma_start(out=raw[:, j, :], in_=cc_out[l][j, :, :])
                    nc.vector.tensor_copy(out=xp16[l][:, j, :],
                                          in_=_rev_view(raw[:, j, :], N))

            # ============ Phase 0: embeddings ============
            with tc.tile_pool(name="x0t", bufs=1) as x0t:
                x0_T = [x0t.tile([128, N], F16, tag="x0t0", name="x0t0"),
                        x0t.tile([128, N], F16, tag="x0t1", name="x0t1"),
                        x0t.tile([128, N], F16, tag="x0t2", name="x0t2")]
                with tc.tile_pool(name="emb", bufs=2) as embp:
                    fill_t(x0_T[2], 0.0, embp)
                    for a in range(6):
                        for c, (c0, cw) in enumerate([(0, 128), (128, 128), (256, 44)]):
                            tp = psum.tile([128, 128], F32, tag="zp", name="tp")
                            nc.tensor.transpose(tp[:cw, :], wrows[:, a, ds(c0, cw)], ident)
                            if c < 2:
                                nc.vector.tensor_copy(out=x0_T[c][:, ts(a, 128)], in_=tp[:cw, :])
                            else:
                                nc.vector.tensor_copy(out=x0_T[2][0:44, ts(a, 128)], in_=tp[:44, :])
                        tp = psum.tile([128, 128], F32, tag="zp", name="tp")
                        nc.tensor.transpose(tp[:EP, :], prows[:, a, :], ident)
                        nc.vector.tensor_copy(out=x0_T[2][64:128, ts(a, 128)], in_=tp[:EP, :])

                # ============ layer 0 ============
                with tc.tile_pool(name="ph0", bufs=1) as ph0:
                    x_pre0 = ph0.tile([128, 16, N], F16, tag="xpre0")
                    for mt in MT_ORDER:
                        zp = psum.tile([128, N], F32, tag="zp")
                        for kk in range(3):
                            for (n0, nw) in NCH:
                                nc.tensor.matmul(
                                    out=zp[:, ds(n0, nw)],
                                    lhsT=wt0[kk][:, ts(mt, 128)],
                                    rhs=x0_T[kk][:, ds(n0, nw)],
                                    start=(kk == 0), stop=(kk == 2))
                        nc.vector.tensor_copy(out=x_pre0[:, mt, :], in_=zp)
                    def _stale_send0(j):
                        send_chunk_buf(h_snap, j, cc_in[0], cc_out[0])

                    def _delta_send0(j):
                        # delta = final - snapshot (overwrite snapshot)
                        nc.vector.tensor_tensor(
                            out=h_snap[:, j, :], in0=own16[0][:, j, :],
                            in1=h_snap[:, j, :], op=OP.subtract)
                        send_chunk_buf(h_snap, j, cc_in_d, cc_out_d)

                    lstm_sweeps(x_pre0, u16_0, u8_0, b_sb[0], own16[0],
                                snap=h_snap, snap_done=_stale_send0,
                                chunk_done=_delta_send0)

            # ===== exchange 0 overlapped with layer-1 own-half x_pre =====
            with tc.tile_pool(name="ph1", bufs=1) as ph1:
                x_pre1 = ph1.tile([128, 16, N], F16, tag="xpre1")
                u16_1 = ph1.tile([128, 4, G4], F16, tag="u16_1", name="u16_1")
                for kk in range(4):
                    nc.sync.dma_start(out=u16_1[:, kk, :],
                                      in_=whh1_16[ds(kk * 128, 128), :])
                u8_1 = ph1.tile([128, 4, G4], F8, tag="u8_1", name="u8_1")
                nc.sync.dma_start(out=u8_1, in_=whh1_8[:, :, :])
                with tc.tile_pool(name="exc0", bufs=1) as exc0:
                    # pass A: own-direction half (rows 512:1024 = wt1[4:8])
                    for mt in MT_ORDER:
                        zp = psum.tile([128, N], F32, tag="zp")
                        for i_kk, kk in enumerate(range(4)):
                            for (n0, nw) in NCH:
                                nc.tensor.matmul(
                                    out=zp[:, ds(n0, nw)],
                                    lhsT=wt1[4 + kk][:, ts(mt, 128)],
                                    rhs=own16[0][:, kk, ds(n0, nw)],
                                    start=(i_kk == 0), stop=(i_kk == 3))
                        nc.vector.tensor_copy(out=x_pre1[:, mt, :], in_=zp)
                    recv(0, exc0)
                    # pass B: partner half accumulated on top
                    for mt in MT_ORDER:
                        zp = psum.tile([128, N], F32, tag="zp")
                        for i_kk, kk in enumerate(range(4)):
                            for (n0, nw) in NCH:
                                nc.tensor.matmul(
                                    out=zp[:, ds(n0, nw)],
                                    lhsT=wt1[kk][:, ts(mt, 128)],
                                    rhs=xp16[0][:, kk, ds(n0, nw)],
                                    start=(i_kk == 0), stop=(i_kk == 3))
                        nc.vector.tensor_tensor(out=x_pre1[:, mt, :], in0=x_pre1[:, mt, :],
                                                in1=zp, op=OP.add)

                # ============ layer 1 ============
                def _hook1(k):
                    if k != 3:
                        return
                    # delta correction: x_pre1 += Wih1_partner @ rev(delta)
                    # (h_snap is dead after the delta send -- reuse as recv buf)
                    for j in range(4):
                        nc.sync.dma_start(out=h_snap[:, j, :], in_=cc_out_d[j, :, :])
                        nc.vector.tensor_copy(out=xp16[0][:, j, :],
                                              in_=_rev_view(h_snap[:, j, :], N))
                    for mt in MT_ORDER:
                        zp = psum.tile([128, N], F32, tag="zp")
                        for i_kk, kk in enumerate(range(4)):
                            for (n0, nw) in NCH:
                                nc.tensor.matmul(
                                    out=zp[:, ds(n0, nw)],
                                    lhsT=wt1[kk][:, ts(mt, 128)],
                                    rhs=xp16[0][:, kk, ds(n0, nw)],
                                    start=(i_kk == 0), stop=(i_kk == 3))
                        nc.vector.tensor_tensor(out=x_pre1[:, mt, :],
                                                in0=x_pre1[:, mt, :],
                                                in1=zp, op=OP.add)

                lstm_sweeps(x_pre1, u16_1, u8_1, b_sb[1], own16[1],
                            chunk_done=lambda j: send_chunk(1, j),
                            sweep_hook=_hook1)

            # ===== exchange 1 + head (th/tm own-half overlapped in PSUM) =====
            with tc.tile_pool(name="head", bufs=1) as hd:
                wtiles = {}
                for wi, w_dram in enumerate((wh_t, wm_t)):
                    for kk in (4, 5, 6, 7, 0, 1, 2, 3):
                        wr = hd.tile([128, M_MLP], F16, tag=f"hw{wi}_{kk}",
                                     name=f"hw{wi}_{kk}")
                        nc.sync.dma_start(out=wr, in_=w_dram[ds(kk * 128, 128), :])
                        wtiles[(wi, kk)] = wr
                at_tiles = []
                for kk, pk in ((0, 128), (1, 128), (2, 1)):
                    wr = hd.tile([128, M_MLP + 1], F16, tag=f"at_r{kk}", name=f"at_r{kk}")
                    nc.sync.dma_start(out=wr[:pk, :], in_=a_t[ds(kk * 128, pk), :])
                    at_tiles.append(wr)
                th_r = [hd.tile([128, N], F16, tag=f"thr{c}", name=f"thr{c}") for c in range(2)]
                tm_r = [hd.tile([128, N], F16, tag=f"tmr{c}", name=f"tmr{c}") for c in range(2)]
                with tc.tile_pool(name="exc1", bufs=1) as exc1:
                    # pass A: own half (rows 512:1024) into held-open PSUM
                    zps = {}
                    for wi in range(2):
                        for mt in range(2):
                            zp = psum.tile([128, N], F32, tag="zp",
                                           name=f"zph{wi}{mt}")
                            zps[(wi, mt)] = zp
                            for i_kk, kk in enumerate(range(4)):
                                for (n0, nw) in NCH:
                                    nc.tensor.matmul(out=zp[:, ds(n0, nw)],
                                                     lhsT=wtiles[(wi, 4 + kk)][:, ts(mt, 128)],
                                                     rhs=own16[1][:, kk, ds(n0, nw)],
                                                     start=(i_kk == 0), stop=False)
                    recv(1, exc1)
                    # pass B: partner half, close accumulation, tanh
                    for wi, (bias_t, dst) in enumerate(((bh_sb, th_r), (bm_sb, tm_r))):
                        for mt in range(2):
                            zp = zps[(wi, mt)]
                            for i_kk, kk in enumerate(range(4)):
                                for (n0, nw) in NCH:
                                    nc.tensor.matmul(out=zp[:, ds(n0, nw)],
                                                     lhsT=wtiles[(wi, kk)][:, ts(mt, 128)],
                                                     rhs=xp16[1][:, kk, ds(n0, nw)],
                                                     start=False, stop=(i_kk == 3))
                            nc.scalar.activation(out=dst[mt], in_=zp, func=AF.Tanh,
                                                 bias=bias_t[:, mt:mt + 1], scale=1.0)

                ones_row = hd.tile([1, N], F16, tag="ones1")
                with tc.tile_pool(name="zf2", bufs=1) as zf2:
                    fill_t(ones_row, 1.0, zf2, shape=[1, N])

                # Q_att = A @ mb_^T
                q_att = [hd.tile([128, N], F16, tag="qa0", name="qa0"),
                         hd.tile([128, N], F16, tag="qa1", name="qa1"),
                         hd.tile([1, N], F16, tag="qa2", name="qa2")]
                rhs_mb = [(tm_r[0], 128), (tm_r[1], 128), (ones_row, 1)]
                for mt, mw in ((0, 128), (1, 128), (2, 1)):
                    zp = psum.tile([128, N], F32, tag="zp")
                    for kk, (rt, pk) in enumerate(rhs_mb):
                        for (n0, nw) in NCH:
                            nc.tensor.matmul(out=zp[:mw, ds(n0, nw)],
                                             lhsT=at_tiles[kk][:pk, ds(mt * 128, mw)],
                                             rhs=rt[:pk, ds(n0, nw)],
                                             start=(kk == 0), stop=(kk == 2))
                    nc.vector.tensor_copy(out=q_att[mt][:mw, :], in_=zp[:mw, :])

                # P/Q Taylor blocks (all fp16: 2x DVE)
                p_mlp = [[hd.tile([128, N], F16, tag=f"pm{p}_{c}", name=f"pm{p}_{c}")
                          for c in range(2)] for p in range(N_PW)]
                q_mlp = [[hd.tile([128, N], F16, tag=f"qm{p}_{c}", name=f"qm{p}_{c}")
                          for c in range(2)] for p in range(N_PW)]
                for c in range(2):
                    wfc = wf_sb[:, c:c + 1]
                    nwfc = negwf_sb[:, c:c + 1]
                    th2 = hd.tile([128, N], F16, tag="th2")
                    nc.vector.tensor_tensor(out=th2, in0=th_r[c], in1=th_r[c], op=OP.mult)
                    negw1 = hd.tile([128, N], F16, tag="negw1")
                    nc.vector.tensor_scalar(out=negw1, in0=th2, scalar1=wfc, scalar2=nwfc,
                                            op0=OP.mult, op1=OP.add)
                    nc.vector.tensor_scalar_mul(p_mlp[0][c], th_r[c], wfc)
                    nc.vector.tensor_scalar(out=p_mlp[1][c], in0=th2, scalar1=nwfc, scalar2=wfc,
                                            op0=OP.mult, op1=OP.add)
                    nc.vector.tensor_tensor(out=p_mlp[2][c], in0=th_r[c], in1=negw1, op=OP.mult)
                    one_t = hd.tile([128, N], F16, tag="one_t")
                    nc.vector.memset(one_t, 1.0)
                    nc.vector.tensor_copy(out=q_mlp[0][c], in_=one_t)
                    nc.vector.tensor_copy(out=q_mlp[1][c], in_=tm_r[c])
                    nc.vector.tensor_tensor(out=q_mlp[2][c], in0=tm_r[c], in1=tm_r[c], op=OP.mult)

                kblocks = [(th_r[0], q_att[0], 128), (th_r[1], q_att[1], 128),
                           (ones_row, q_att[2], 1)]
                for p in range(N_PW):
                    for c in range(2):
                        kblocks.append((p_mlp[p][c], q_mlp[p][c], 128))
                nkb = len(kblocks)
                for xt in range(6):
                    zp = psum.tile([128, N], F32, tag="zp")
                    for kb, (pt, qt, pk) in enumerate(kblocks):
                        for (n0, nw) in NCH:
                            nc.tensor.matmul(out=zp[:, ds(n0, nw)],
                                             lhsT=pt[:pk, ts(xt, 128)],
                                             rhs=qt[:pk, ds(n0, nw)],
                                             start=(kb == 0), stop=(kb == nkb - 1))
                    srow = hd.tile([128, N], F16, tag="srow", bufs=2)
                    nc.scalar.activation(out=srow, in_=zp, func=AF.Identity,
                                         bias=bf_sb, scale=1.0)
                    nc.sync.dma_start(out=scores[ts(xt, 128), :], in_=srow)

    nc.finalize()
    return nc


_NC_CACHE = {}


def _get_module():
    key = (N_F8, N_F16, N_PW)
    if key not in _NC_CACHE:
        _NC_CACHE[key] = build_module()
    return _NC_CACHE[key]


def _pad_wih0(wt):
    """[364, G4] -> [384, G4]: word rows 0:300, zeros, pos rows at 320:384."""
    pad = np.zeros((DIN0, wt.shape[1]), np.float32)
    pad[0:300] = wt[0:300]
    pad[320:384] = wt[300:364]
    return pad


def _to_f8(w):
    return np.ascontiguousarray(w.astype(ml_dtypes.float8_e4m3))


def _whh8(whh_t):
    """[512, 2048] f32 (Whh.T) -> [128, 4, 2048] fp8 (k-tile layout)."""
    w = np.asarray(whh_t, np.float32).reshape(4, 128, G4).transpose(1, 0, 2)
    return _to_f8(w)


def _prep_inputs_core(inputs, core):
    f32, f16 = np.float32, np.float16
    is_f = core < 4
    d = "f" if is_f else "b"
    widx = np.asarray(inputs["word_idx"]).reshape(-1).astype(np.int32)
    pidx = np.asarray(inputs["pos_idx"]).reshape(-1).astype(np.int32)
    if not is_f:
        widx = widx[::-1]
        pidx = pidx[::-1]
    wih1 = np.asarray(inputs[f"Wih1{d}"]).T.astype(f32)     # [1024, 2048]
    wh = np.asarray(inputs["Wh"]).T.astype(f32)             # [1024, 256]
    wm = np.asarray(inputs["Wm"]).T.astype(f32)
    if is_f:
        # program's x order is [partner(=b); own(=f)] -> permute rows
        wih1 = np.concatenate([wih1[512:1024], wih1[0:512]], 0)
        wh = np.concatenate([wh[512:1024], wh[0:512]], 0)
        wm = np.concatenate([wm[512:1024], wm[0:512]], 0)
    whh0_t = np.asarray(inputs[f"Whh0{d}"]).T.astype(f32)   # [512, 2048]
    whh1_t = np.asarray(inputs[f"Whh1{d}"]).T.astype(f32)
    # scatter own h into the slot of the PARTNER's rank (fwd rank 0, bwd 1)
    rank = 0 if is_f else 1
    sidx_arr = np.stack(
        [((1 - rank) * 4 + j) * 128 + np.arange(128) for j in range(4)],
        axis=1).astype(np.int32)
    im = {
        "widx": np.ascontiguousarray(widx),
        "pidx": np.ascontiguousarray(pidx),
        "wemb": np.ascontiguousarray(inputs["word_emb"], dtype=f32),
        "pemb": np.ascontiguousarray(inputs["pos_emb"], dtype=f32),
        "wih0_t": np.ascontiguousarray(
            _pad_wih0(np.asarray(inputs[f"Wih0{d}"]).T.astype(f32)).astype(f16)),
        "whh0_16": np.ascontiguousarray(whh0_t.astype(f16)),
        "whh0_8": _whh8(whh0_t),
        "b0": np.ascontiguousarray(inputs[f"b0{d}"], dtype=f32),
        "wih1_t": np.ascontiguousarray(wih1.astype(f16)),
        "whh1_16": np.ascontiguousarray(whh1_t.astype(f16)),
        "whh1_8": _whh8(whh1_t),
        "b1": np.ascontiguousarray(inputs[f"b1{d}"], dtype=f32),
        "wh_t": np.ascontiguousarray(wh.astype(f16)),
        "wm_t": np.ascontiguousarray(wm.astype(f16)),
        "bh": np.ascontiguousarray(inputs["bh"], dtype=f32),
        "bm": np.ascontiguousarray(inputs["bm"], dtype=f32),
        "a_t": np.ascontiguousarray(np.asarray(inputs["A"])[0].T.astype(f16)),
        "wf": np.ascontiguousarray(np.asarray(inputs["Wf"]).reshape(-1), dtype=f32),
        "bf": np.ascontiguousarray(np.asarray(inputs["bf"]).reshape(-1), dtype=f32),
        "sidx": sidx_arr,
    }
    return im


_RUNNER_CACHE = {}


def _get_runner():
    """Cached jitted 8-core runner (mirrors bass2jax.run_bass_via_pjrt's
    multi-core path, but reuses the compiled executable across calls)."""
    key = (N_F8, N_F16, N_PW)
    if key in _RUNNER_CACHE:
        return _RUNNER_CACHE[key]
    import jax
    from jax.sharding import Mesh, PartitionSpec
    from jax.experimental.shard_map import shard_map
    from concourse.bass2jax import (_bass_exec_p, install_neuronx_cc_hook,
                                    partition_id_tensor)
    nc = _get_module()
    install_neuronx_cc_hook()
    partition_name = nc.partition_id_tensor.name if nc.partition_id_tensor else None
    in_names, out_names, out_avals, zero_shapes = [], [], [], []
    for alloc in nc.m.functions[0].allocations:
        if not isinstance(alloc, mybir.MemoryLocationSet):
            continue
        name = alloc.memorylocations[0].name
        if alloc.kind == "ExternalInput":
            if name != partition_name:
                in_names.append(name)
        elif alloc.kind == "ExternalOutput":
            shape = tuple(alloc.tensor_shape)
            dtype = mybir.dt.np(alloc.dtype)
            out_avals.append(jax.core.ShapedArray(shape, dtype))
            out_names.append(name)
            zero_shapes.append((shape, dtype))
    n_params, n_outs = len(in_names), len(out_names)
    full_in_names = list(in_names) + list(out_names)
    if partition_name is not None:
        full_in_names.append(partition_name)
    donate = tuple(range(n_params, n_params + n_outs))

    def _body(*args):
        operands = list(args)
        if partition_name is not None:
            operands.append(partition_id_tensor())
        outs = _bass_exec_p.bind(
            *operands, out_avals=tuple(out_avals), in_names=tuple(full_in_names),
            out_names=tuple(out_names), lowering_input_output_aliases=(),
            sim_require_finite=True, sim_require_nnan=True, nc=nc)
        return tuple(outs)

    devices = jax.devices()[:N_CORES]
    mesh = Mesh(np.asarray(devices), ("core",))
    sharded = jax.jit(
        shard_map(_body, mesh=mesh,
                  in_specs=(PartitionSpec("core"),) * (n_params + n_outs),
                  out_specs=(PartitionSpec("core"),) * n_outs,
                  check_rep=False),
        donate_argnums=donate, keep_unused=True)

    def run(ims):
        concat_in = [np.concatenate([np.asarray(ims[c][nm]) for c in range(N_CORES)], 0)
                     for nm in in_names]
        concat_zeros = [np.zeros((N_CORES * sh[0], *sh[1:]), dt)
                        for sh, dt in zero_shapes]
        out_arrs = sharded(*concat_in, *concat_zeros)
        return [{nm: np.asarray(out_arrs[i]).reshape(N_CORES, *out_avals[i].shape)[c]
                 for i, nm in enumerate(out_names)} for c in range(N_CORES)]

    _RUNNER_CACHE[key] = run
    return run


def kernel(**inputs) -> np.ndarray:
    inputs = {k: np.asarray(v) for k, v in inputs.items()}
    run = _get_runner()
    ims = [_prep_inputs_core(inputs, c) for c in range(N_CORES)]
    results = run(ims)
    out = np.asarray(results[0]["scores"], dtype=np.float32)
    return np.ascontiguousarray(out.reshape(1, N, N))


def run_debug(inputs, cores=(0,)):
    nc = _get_module()
    inputs = {k: np.asarray(v) for k, v in inputs.items()}
    ims = [_prep_inputs_core(inputs, c) for c in range(N_CORES)]
    res = run_bass_kernel_spmd(nc, ims, core_ids=list(range(N_CORES)))
    return [res.results[c] for c in cores]
